# revision 12
# baseline (speedup 1.0000x reference)
"""HONU order-3 kernel for 8 TRN2 NeuronCores.

Math: out[b] = sum_{i<=j<=k} w_ijk * xf_i * xf_j * xf_k,  xf = [1, x] (127 feats).

Restructuring: group combos by pair (i,j) (lex order => per-pair weights are a
contiguous slice of `weights`).  Let W[(i,j), k] = w_ijk for k>=j (0 otherwise).
Then  Z[b,(i,j)] = sum_k W[(i,j),k] * xf[b,k]   (a dense matmul), and
      out[b]     = sum_{(i,j)} Q[b,(i,j)] * Z[b,(i,j)],   Q[b,(i,j)] = xf_i*xf_j.

Sharding: pair-rows i are dealt round-robin to the 8 cores (core c gets rows
i = 8t + c, t = 0..15); class t occupies columns [OFFS[t], OFFS[t+1]) covering
j in [8t, 128).  NCOLS = 1088 per core.  Q is built on the host as INT8 with a
per-column scale folded into the bf16 weights.

Schedule (latency-driven):
 - All inputs ship as TWO byte-blob dram tensors with wide rows (2688B /
   2176B) so each HWDGE queue streams ~140-150GB/s; 4 triggers total,
   64/64 partition-split across the SP and ACT queues (64-aligned splits
   matter: a 76/52 split doubled the ACT trigger cost), xt+W first (gates
   the matmuls), Q second.  bf16 data rides in int8 blobs, bitcast on SBUF.
 - The Q*Z multiply-reduce runs as one scalar_tensor_tensor per batch-tile
   on DVE, reading Z straight from PSUM, accumulating into adjacent columns
   of one [128,4] f32 tile -> ONE output DMA (no transpose, no merge; the
   host ignores the 2 spare columns).  PE warm-up trains were tried and
   measurably HURT (queue-drain delay > p-state gain) - do not re-add.
x is replicated; each core returns [128,4] partials the host reduces.
"""

import numpy as np
import ml_dtypes

import concourse.bass as bass
import concourse.bacc as bacc
import concourse.tile as tile
import concourse.mybir as mybir
from concourse.bass_utils import run_bass_kernel_spmd

F32 = mybir.dt.float32
BF16 = mybir.dt.bfloat16
I8 = mybir.dt.int8
BF16_NP = ml_dtypes.bfloat16

P = 128
NF = 127            # features incl. bias
B = 256             # batch
NCLASS = 16
WIDTHS = [128 - 8 * t for t in range(NCLASS)]
OFFS = np.concatenate([[0], np.cumsum(WIDTHS)])
NCOLS = int(OFFS[-1])                                   # 1088

_CACHE = {}


def _build_nc():
    nc = bacc.Bacc("TRN2", target_bir_lowering=False, debug=False)
    # byte blobs: [xt bf16 512B | W bf16 2176B] rows=feature; [qh0|qh1] rows=batch
    xw = nc.dram_tensor("xw", [P, 512 + 2 * NCOLS], I8, kind="ExternalInput")
    qq = nc.dram_tensor("qq", [P, 2 * NCOLS], I8, kind="ExternalInput")
    out = nc.dram_tensor("out", [P, 4], F32, kind="ExternalOutput")

    with tile.TileContext(nc) as tc:
        with (
            tc.tile_pool(name="const", bufs=1) as cpool,
            tc.tile_pool(name="ps", bufs=1, space="PSUM") as ps,
        ):
            xw_t = cpool.tile([P, 512 + 2 * NCOLS], I8, tag="xw")
            qq_t = cpool.tile([P, 2 * NCOLS], I8, tag="qq")
            # 4 input triggers: xt+W halves first (gates PE), Q halves second
            nc.sync.dma_start(xw_t[0:64, :], xw[0:64, :])
            nc.scalar.dma_start(xw_t[64:P, :], xw[64:P, :])
            nc.sync.dma_start(qq_t[0:64, :], qq[0:64, :])
            nc.scalar.dma_start(qq_t[64:P, :], qq[64:P, :])

            xt = xw_t[:, 0:512].bitcast(BF16)               # [128, 256]
            wv = xw_t[:, 512:512 + 2 * NCOLS].bitcast(BF16)  # [128, 1088]
            qh = [qq_t[:, 0:NCOLS], qq_t[:, NCOLS:2 * NCOLS]]

            res4 = cpool.tile([P, 4], F32, tag="res4")
            eD = cpool.tile([P, NCOLS], F32, tag="eD")
            zb1 = cpool.tile([P, NCOLS], BF16, tag="zb1")
            zs = []
            for bt in range(2):
                z_ps = ps.tile([P, NCOLS], F32, tag=f"z{bt}")
                xts = xt[:, bt * P:(bt + 1) * P]
                nc.tensor.matmul(z_ps[:, 0:512], xts, wv[:, 0:512],
                                 start=True, stop=True)
                nc.tensor.matmul(z_ps[:, 512:1024], xts, wv[:, 512:1024],
                                 start=True, stop=True)
                nc.tensor.matmul(z_ps[:, 1024:NCOLS], xts, wv[:, 1024:NCOLS],
                                 start=True, stop=True)
                zs.append(z_ps)

            # bt1's Z staged to SBUF bf16 by the idle ACT engine, chunk by
            # chunk in STT0's shadow, so STT1 reads all-SBUF operands (DVE
            # 2x-eligible); bt0 reads PSUM directly (a copy would gate it).
            for lo, hi in ((0, 512), (512, 1024), (1024, NCOLS)):
                nc.scalar.copy(zb1[:, lo:hi], zs[1][:, lo:hi])
            nc.vector.scalar_tensor_tensor(
                out=eD[:], in0=zs[0][:], scalar=1.0, in1=qh[0][:],
                op0=mybir.AluOpType.mult, op1=mybir.AluOpType.mult,
                accum_out=res4[:, 0:1],
            )
            nc.vector.scalar_tensor_tensor(
                out=eD[:], in0=zb1[:], scalar=1.0, in1=qh[1][:],
                op0=mybir.AluOpType.mult, op1=mybir.AluOpType.mult,
                accum_out=res4[:, 1:2],
            )

            nc.scalar.dma_start(out[:, :], res4[:])
    nc.compile()
    return nc


def _prep_inputs(x, weights, comb_idx):
    """Host-side layout prep: byte blobs per core (bf16 xt+W, int8 Q with
    per-column scale folded into the bf16 weight columns)."""
    x = np.ascontiguousarray(np.asarray(x, dtype=np.float32))
    w = np.asarray(weights, dtype=np.float32).ravel()
    ci = np.asarray(comb_idx)
    i_, j_ = ci[:, 0].astype(np.int64), ci[:, 1].astype(np.int64)
    k_ = ci[:, 2].astype(np.int64)

    xf = np.concatenate([np.ones((B, 1), np.float32), x], axis=1)   # [256,127]
    xbp = np.zeros((B, P), np.float32)
    xbp[:, :NF] = xf

    xt = np.zeros((P, B), np.float32)
    xt[:NF, :] = xf.T
    xt16 = xt.astype(BF16_NP)

    # lex pair-row index of each combo
    ar = np.arange(NF, dtype=np.int64)
    rsp = ar * NF - (ar * (ar - 1)) // 2
    q = rsp[i_] + (j_ - i_)
    Wd = np.zeros((8128, NF), np.float32)
    Wd[q, k_] = w

    in_maps = []
    for c in range(8):
        big = np.zeros((P, NCOLS), np.float32)
        Q = np.zeros((B, NCOLS), np.float32)
        for t in range(NCLASS):
            i = 8 * t + c
            if i > 126:
                continue
            o = int(OFFS[t])
            Q[:, o:o + WIDTHS[t]] = xf[:, i:i + 1] * xbp[:, 8 * t:P]
            p0 = int(rsp[i])
            big[:NF, o + (i - 8 * t): o + (NF - 8 * t)] = Wd[p0:p0 + (NF - i)].T
        # int8 quantization of Q with per-column scale folded into weights
        scale = np.abs(Q).max(0) / 127.0
        scale[scale == 0] = 1.0
        Q8 = np.clip(np.round(Q / scale), -127, 127).astype(np.int8)
        big16 = (big * scale[None, :]).astype(BF16_NP)
        xw_blob = np.concatenate(
            [xt16.view(np.uint8), big16.view(np.uint8)], axis=1
        ).view(np.int8)
        qq_blob = np.ascontiguousarray(
            np.concatenate([Q8[0:P], Q8[P:B]], axis=1))
        in_maps.append({"xw": np.ascontiguousarray(xw_blob), "qq": qq_blob})
    return in_maps


def _get_nc():
    if "nc" not in _CACHE:
        _CACHE["nc"] = _build_nc()
    return _CACHE["nc"]


def run_spmd(x, weights, comb_idx, trace=False):
    nc = _get_nc()
    in_maps = _prep_inputs(x, weights, comb_idx)
    res = run_bass_kernel_spmd(nc, in_maps, list(range(8)), trace=trace)
    acc = np.zeros((P, 4), np.float64)
    for c in range(8):
        acc += res.results[c]["out"].astype(np.float64)
    full = np.concatenate([acc[:, 0], acc[:, 1]])
    return full.reshape(B, 1).astype(np.float32), res


def kernel(x, weights, comb_idx):
    out, _ = run_spmd(x, weights, comb_idx, trace=False)
    return out


# revision 13
# speedup vs baseline: 1.0461x; 1.0461x over previous
"""HONU order-3 kernel for 8 TRN2 NeuronCores.

Math: out[b] = sum_{i<=j<=k} w_ijk * xf_i * xf_j * xf_k,  xf = [1, x] (127 feats).

Restructuring: group combos by pair (i,j) (lex order => per-pair weights are a
contiguous slice of `weights`).  Let W[(i,j), k] = w_ijk for k>=j (0 otherwise).
Then  Z[b,(i,j)] = sum_k W[(i,j),k] * xf[b,k]   (a dense matmul), and
      out[b]     = sum_{(i,j)} Q[b,(i,j)] * Z[b,(i,j)],   Q[b,(i,j)] = xf_i*xf_j.

Sharding: pair-rows i are dealt round-robin to the 8 cores (core c gets rows
i = 8t + c, t = 0..15); class t occupies columns [OFFS[t], OFFS[t+1]) covering
j in [8t, 128).  NCOLS = 1088 per core.  Q is built on the host as INT8 with a
per-column scale folded into the bf16 weights.

Schedule (latency-driven):
 - All inputs ship as TWO byte-blob dram tensors with wide rows (2688B /
   2176B) so each HWDGE queue streams ~140-150GB/s; 4 triggers total,
   64/64 partition-split across the SP and ACT queues (64-aligned splits
   matter: a 76/52 split doubled the ACT trigger cost), xt+W first (gates
   the matmuls), Q second.  bf16 data rides in int8 blobs, bitcast on SBUF.
 - The Q*Z multiply-reduce runs as one scalar_tensor_tensor per batch-tile
   on DVE, reading Z straight from PSUM, accumulating into adjacent columns
   of one [128,4] f32 tile -> ONE output DMA (no transpose, no merge; the
   host ignores the 2 spare columns).  PE warm-up trains were tried and
   measurably HURT (queue-drain delay > p-state gain) - do not re-add.
x is replicated; each core returns [128,4] partials the host reduces.
"""

import numpy as np
import ml_dtypes

import concourse.bass as bass
import concourse.bacc as bacc
import concourse.tile as tile
import concourse.mybir as mybir
from concourse.bass_utils import run_bass_kernel_spmd

F32 = mybir.dt.float32
BF16 = mybir.dt.bfloat16
I8 = mybir.dt.int8
BF16_NP = ml_dtypes.bfloat16

P = 128
NF = 127            # features incl. bias
B = 256             # batch
NCLASS = 16
WIDTHS = [128 - 8 * t for t in range(NCLASS)]
OFFS = np.concatenate([[0], np.cumsum(WIDTHS)])
NCOLS = int(OFFS[-1])                                   # 1088

_CACHE = {}


def _build_nc():
    nc = bacc.Bacc("TRN2", target_bir_lowering=False, debug=False)
    # byte blobs: [xt bf16 512B | W bf16 2176B] rows=feature; [qh0|qh1] rows=batch
    xw = nc.dram_tensor("xw", [P, 512 + 2 * NCOLS], I8, kind="ExternalInput")
    qq = nc.dram_tensor("qq", [P, 2 * NCOLS], I8, kind="ExternalInput")
    out = nc.dram_tensor("out", [P, 4], F32, kind="ExternalOutput")

    with tile.TileContext(nc) as tc:
        with (
            tc.tile_pool(name="const", bufs=1) as cpool,
            tc.tile_pool(name="ps", bufs=1, space="PSUM") as ps,
        ):
            xw_t = cpool.tile([P, 512 + 2 * NCOLS], I8, tag="xw")
            qq_t = cpool.tile([P, 2 * NCOLS], I8, tag="qq")
            # 4 input triggers: xt+W halves first (gates PE), Q halves second
            nc.sync.dma_start(xw_t[0:64, :], xw[0:64, :])
            nc.scalar.dma_start(xw_t[64:P, :], xw[64:P, :])
            nc.sync.dma_start(qq_t[0:64, :], qq[0:64, :])
            nc.scalar.dma_start(qq_t[64:P, :], qq[64:P, :])

            xt = xw_t[:, 0:512].bitcast(BF16)               # [128, 256]
            wv = xw_t[:, 512:512 + 2 * NCOLS].bitcast(BF16)  # [128, 1088]
            qh = [qq_t[:, 0:NCOLS], qq_t[:, NCOLS:2 * NCOLS]]

            res4 = cpool.tile([P, 4], F32, tag="res4")
            eD = cpool.tile([P, NCOLS], F32, tag="eD")
            zs = []
            for bt in range(2):
                z_ps = ps.tile([P, NCOLS], F32, tag=f"z{bt}")
                xts = xt[:, bt * P:(bt + 1) * P]
                nc.tensor.matmul(z_ps[:, 0:512], xts, wv[:, 0:512],
                                 start=True, stop=True)
                nc.tensor.matmul(z_ps[:, 512:1024], xts, wv[:, 512:1024],
                                 start=True, stop=True)
                nc.tensor.matmul(z_ps[:, 1024:NCOLS], xts, wv[:, 1024:NCOLS],
                                 start=True, stop=True)
                zs.append(z_ps)

            for bt in range(2):
                # fused multiply+reduce over all 1088 cols straight from PSUM
                nc.vector.scalar_tensor_tensor(
                    out=eD[:], in0=zs[bt][:], scalar=1.0,
                    in1=qh[bt][:],
                    op0=mybir.AluOpType.mult, op1=mybir.AluOpType.mult,
                    accum_out=res4[:, bt:bt + 1],
                )

            nc.scalar.dma_start(out[:, :], res4[:])
    nc.compile()
    return nc


def _prep_inputs(x, weights, comb_idx):
    """Host-side layout prep: byte blobs per core (bf16 xt+W, int8 Q with
    per-column scale folded into the bf16 weight columns)."""
    x = np.ascontiguousarray(np.asarray(x, dtype=np.float32))
    w = np.asarray(weights, dtype=np.float32).ravel()
    ci = np.asarray(comb_idx)
    i_, j_ = ci[:, 0].astype(np.int64), ci[:, 1].astype(np.int64)
    k_ = ci[:, 2].astype(np.int64)

    xf = np.concatenate([np.ones((B, 1), np.float32), x], axis=1)   # [256,127]
    xbp = np.zeros((B, P), np.float32)
    xbp[:, :NF] = xf

    xt = np.zeros((P, B), np.float32)
    xt[:NF, :] = xf.T
    xt16 = xt.astype(BF16_NP)

    # lex pair-row index of each combo
    ar = np.arange(NF, dtype=np.int64)
    rsp = ar * NF - (ar * (ar - 1)) // 2
    q = rsp[i_] + (j_ - i_)
    Wd = np.zeros((8128, NF), np.float32)
    Wd[q, k_] = w

    in_maps = []
    for c in range(8):
        big = np.zeros((P, NCOLS), np.float32)
        Q = np.zeros((B, NCOLS), np.float32)
        for t in range(NCLASS):
            i = 8 * t + c
            if i > 126:
                continue
            o = int(OFFS[t])
            Q[:, o:o + WIDTHS[t]] = xf[:, i:i + 1] * xbp[:, 8 * t:P]
            p0 = int(rsp[i])
            big[:NF, o + (i - 8 * t): o + (NF - 8 * t)] = Wd[p0:p0 + (NF - i)].T
        # int8 quantization of Q with per-column scale folded into weights
        scale = np.abs(Q).max(0) / 127.0
        scale[scale == 0] = 1.0
        Q8 = np.clip(np.round(Q / scale), -127, 127).astype(np.int8)
        big16 = (big * scale[None, :]).astype(BF16_NP)
        xw_blob = np.concatenate(
            [xt16.view(np.uint8), big16.view(np.uint8)], axis=1
        ).view(np.int8)
        qq_blob = np.ascontiguousarray(
            np.concatenate([Q8[0:P], Q8[P:B]], axis=1))
        in_maps.append({"xw": np.ascontiguousarray(xw_blob), "qq": qq_blob})
    return in_maps


def _get_nc():
    if "nc" not in _CACHE:
        _CACHE["nc"] = _build_nc()
    return _CACHE["nc"]


def run_spmd(x, weights, comb_idx, trace=False):
    nc = _get_nc()
    in_maps = _prep_inputs(x, weights, comb_idx)
    res = run_bass_kernel_spmd(nc, in_maps, list(range(8)), trace=trace)
    acc = np.zeros((P, 4), np.float64)
    for c in range(8):
        acc += res.results[c]["out"].astype(np.float64)
    full = np.concatenate([acc[:, 0], acc[:, 1]])
    return full.reshape(B, 1).astype(np.float32), res


def kernel(x, weights, comb_idx):
    out, _ = run_spmd(x, weights, comb_idx, trace=False)
    return out
